# revision 33
# baseline (speedup 1.0000x reference)
"""AttentionV1 Trainium2 Bass kernel (v5 = v2 + TS/TT stencil + fused casts).

Data-parallel over batch: 8 images -> 8 NeuronCores. Per core:
  qkv = W_qkv @ x            (1x1 conv, PE, bf16, permuted 5-block layout)
  qkv = dwconv3x3(qkv)       (q,k: tensor_scalar products + tensor_tensor
                              adds on DVE; v: diag-matmul on PE)
  qf = q*f, kf = k*f         (DVE)
  G  = qf @ kf^T             (PE transpose-via-identity + PE gram)
  attn = softmax(G / (nq nk^T))  (small-tensor phase)
  out = (blockdiag(attn)^T @ W_proj^T)^T @ v   (PE)

Channel blocks (output-channel permutation of W_qkv/taps):
  B0 = q[0:128], B1 = q[128:192] || k[128:192], B2 = k[0:128],
  B3 = v[0:128], B4 = v[128:192]
"""
import sys

for _p in ("/opt/trn_rl_repo",):
    if _p not in sys.path:
        sys.path.insert(0, _p)

import numpy as np

import concourse.bass as bass
import concourse.bacc as bacc
import concourse.mybir as mybir
from concourse.tile import TileContext
from concourse.bass_utils import run_bass_kernel_spmd

F32 = mybir.dt.float32
BF16 = mybir.dt.bfloat16
AL = mybir.AluOpType
AF = mybir.ActivationFunctionType

C = 192          # channels
O = 576          # 3*C
H = 128
W = 128
N = H * W        # 16384
HEADS = 8
CH = 24          # channels per head
TR = 8           # rows per spatial tile
NT = H // TR     # 16 tiles
S = TR * W       # 1024 spatial elems per tile
PR = TR + 2      # padded rows (halo)
PW = W + 4       # padded width: cols [2,130) hold x in [0,128)
NCHUNK = S // 128  # 8 transpose chunks per tile

BLK = [128, 128, 128, 128, 64]
QK_BLOCKS = (0, 1, 2)
TAP_OFF = [(3 * (dy + 1) + (dx + 1), dy, dx)
           for dy in (-1, 0, 1) for dx in (-1, 0, 1)]
DVE_TAPS = [(1, -1, 0)] + [t for t in TAP_OFF if t[0] != 1]


def build_nc():
    nc = bacc.Bacc()
    x_d = nc.declare_dram_parameter("x", [C, H, W], F32, isOutput=False)
    f_d = nc.declare_dram_parameter("f", [C, H, W], F32, isOutput=False)
    wq_d = nc.declare_dram_parameter("wq", [C, O], BF16, isOutput=False)
    taps_d = nc.declare_dram_parameter("taps", [O, 9], F32, isOutput=False)
    vd3_d = nc.declare_dram_parameter("vdiag3", [128, 9 * 128], BF16, isOutput=False)
    vd4_d = nc.declare_dram_parameter("vdiag4", [64, 9 * 64], BF16, isOutput=False)
    wp_d = nc.declare_dram_parameter("wp", [C, C], BF16, isOutput=False)
    temp_d = nc.declare_dram_parameter("temp", [CH, HEADS], F32, isOutput=False)
    idb_d = nc.declare_dram_parameter("identb", [128, 128], BF16, isOutput=False)
    idf_d = nc.declare_dram_parameter("identf", [128, 128], F32, isOutput=False)
    out_d = nc.declare_dram_parameter("out", [C, N], BF16, isOutput=True)

    with TileContext(nc) as tc:
        with (
            tc.tile_pool(name="const", bufs=1) as cpool,
            tc.tile_pool(name="vstore", bufs=1) as vpool,
            tc.tile_pool(name="xin", bufs=3) as xpool,
            tc.tile_pool(name="fin", bufs=2) as fpool,
            tc.tile_pool(name="qkv", bufs=3) as qkvpool,
            tc.tile_pool(name="st", bufs=3) as stpool,
            tc.tile_pool(name="scr", bufs=2) as scrpool,
            tc.tile_pool(name="tsb", bufs=3) as tsbpool,
            tc.tile_pool(name="fin2", bufs=1) as finpool,
            tc.tile_pool(name="outsb", bufs=3) as outpool,
            tc.tile_pool(name="mm", bufs=2, space="PSUM") as mmpsum,
            tc.tile_pool(name="vps", bufs=1, space="PSUM") as vpsum,
            tc.tile_pool(name="tps", bufs=1, space="PSUM") as tpsum,
            tc.tile_pool(name="gram", bufs=1, space="PSUM") as gpsum,
        ):
            # ---- constants ----
            wq_sb = [cpool.tile([128, O], BF16, tag="wq0", name="wq0"),
                     cpool.tile([64, O], BF16, tag="wq1", name="wq1")]
            nc.sync.dma_start(out=wq_sb[0][:], in_=wq_d[0:128, :])
            nc.sync.dma_start(out=wq_sb[1][:], in_=wq_d[128:192, :])
            taps_sb = []
            ms = 0
            for bi, psz in enumerate(BLK):
                tt = cpool.tile([psz, 9], F32, tag=f"taps{bi}", name=f"taps{bi}")
                nc.sync.dma_start(out=tt[:], in_=taps_d[ms:ms + psz, :])
                taps_sb.append(tt)
                ms += psz
            vd3 = cpool.tile([128, 9 * 128], BF16, tag="vd3", name="vd3")
            nc.sync.dma_start(out=vd3[:], in_=vd3_d[:])
            vd4 = cpool.tile([64, 9 * 64], BF16, tag="vd4", name="vd4")
            nc.sync.dma_start(out=vd4[:], in_=vd4_d[:])
            wp_sb = [cpool.tile([96, C], BF16, tag="wp0", name="wp0"),
                     cpool.tile([96, C], BF16, tag="wp1", name="wp1")]
            nc.sync.dma_start(out=wp_sb[0][:], in_=wp_d[0:96, :])
            nc.sync.dma_start(out=wp_sb[1][:], in_=wp_d[96:192, :])
            temp_sb = cpool.tile([CH, HEADS], F32, tag="temp", name="temp")
            nc.sync.dma_start(out=temp_sb[:], in_=temp_d[:])
            identb = cpool.tile([128, 128], BF16, tag="identb", name="identb")
            nc.sync.dma_start(out=identb[:], in_=idb_d[:])
            identf = cpool.tile([128, 128], F32, tag="identf", name="identf")
            nc.sync.dma_start(out=identf[:], in_=idf_d[:])

            v_sb = [vpool.tile([128, N], BF16, tag="v0", name="v0"),
                    vpool.tile([64, N], BF16, tag="v1", name="v1")]
            sq_sb = [cpool.tile([128, NT], F32, tag=f"sq{i}", name=f"sq{i}")
                     for i in range(3)]
            # gram accumulators packed into one PSUM bank
            g_all = gpsum.tile([128, 512], F32, tag="g", name="g")
            g_ps = [g_all[:, 0:C], g_all[0:64, 256:256 + C]]

            def emit_tail(t, st, sb):
                """Transposes + gram + v-stencil for tile t. Emitted AFTER
                tile t+1's qkv matmuls so the PE's in-order queue overlaps
                this (DVE/ACT-gated) work with the next tile's independent
                qkv streams."""
                vps = {}
                for j in range(NCHUNK):
                    g = t * NCHUNK + j
                    col = slice(j * 128, (j + 1) * 128)
                    qt_ps = tpsum.tile([128, C], F32, tag="qt", name="qt")
                    kt_ps = tpsum.tile([128, C], F32, tag="kt", name="kt")
                    nc.tensor.matmul(qt_ps[:, 0:128], st[0][:, col],
                                     identb[:], start=True, stop=True)
                    nc.tensor.matmul(qt_ps[:, 128:192], st[1][0:64, col],
                                     identb[0:64, 0:64], start=True, stop=True)
                    nc.tensor.matmul(kt_ps[:, 0:128], st[2][:, col],
                                     identb[:], start=True, stop=True)
                    nc.tensor.matmul(kt_ps[:, 128:192], st[1][64:128, col],
                                     identb[64:128, 64:128], start=True, stop=True)
                    qt_sb = tsbpool.tile([128, C], BF16, tag="qts", name="qts")
                    kt_sb = tsbpool.tile([128, C], BF16, tag="kts", name="kts")
                    nc.scalar.activation(qt_sb[:], qt_ps[:], AF.Copy)
                    nc.scalar.activation(kt_sb[:], kt_ps[:], AF.Copy)
                    half = j // 4
                    jj = j % 4
                    if jj == 0:
                        vps["vp3"] = vpsum.tile([128, 512], F32, tag="vp3",
                                                name="vp3")
                        vps["vp4"] = vpsum.tile([64, 512], F32, tag="vp4",
                                                name="vp4")
                    tap_sl = ((0, 2), (2, 4), (4, 6), (6, 9))[jj]
                    for ti in range(tap_sl[0], tap_sl[1]):
                        _, dy, dx = TAP_OFF[ti]
                        r_lo = 1 + dy + 4 * half
                        rhs3 = sb[3][:, r_lo:r_lo + 4, 2 + dx:2 + dx + W]
                        nc.tensor.matmul(
                            vps["vp3"][:], vd3[:, ti * 128:(ti + 1) * 128],
                            rhs3, start=(ti == 0), stop=(ti == 8))
                        rhs4 = sb[4][:, r_lo:r_lo + 4, 2 + dx:2 + dx + W]
                        nc.tensor.matmul(
                            vps["vp4"][:], vd4[:, ti * 64:(ti + 1) * 64],
                            rhs4, start=(ti == 0), stop=(ti == 8))
                    if jj == 3:
                        cdst = slice(t * S + half * 512,
                                     t * S + (half + 1) * 512)
                        nc.scalar.activation(v_sb[0][:, cdst], vps["vp3"][:],
                                             AF.Copy)
                        nc.scalar.activation(v_sb[1][:, cdst], vps["vp4"][:],
                                             AF.Copy)
                    nc.tensor.matmul(
                        g_ps[0], qt_sb[:, 0:128], kt_sb[:],
                        start=(g == 0), stop=(g == NT * NCHUNK - 1))
                    nc.tensor.matmul(
                        g_ps[1], qt_sb[:, 128:192], kt_sb[:],
                        start=(g == 0), stop=(g == NT * NCHUNK - 1))

            prev = None
            for t in range(NT):
                r0 = t * TR
                xt = [xpool.tile([128, PR * W], BF16, tag="x0", name="x0"),
                      xpool.tile([64, PR * W], BF16, tag="x1", name="x1")]
                lo = r0 - 1
                hi = r0 + TR + 1
                dlo = max(lo, 0)
                dhi = min(hi, H)
                off = dlo - lo
                for ci, (cs, cp) in enumerate(((0, 128), (128, 64))):
                    if lo < 0:
                        nc.vector.memset(xt[ci][:, 0:W], 0.0)
                    if hi > H:
                        nc.vector.memset(xt[ci][:, (PR - 1) * W:PR * W], 0.0)
                    nc.gpsimd.dma_start(
                        out=xt[ci][:, off * W:(off + dhi - dlo) * W],
                        in_=x_d[cs:cs + cp, dlo:dhi, :],
                    )
                ft_a = fpool.tile([128, S], BF16, tag="fa", name="fa")
                nc.gpsimd.dma_start(out=ft_a[:], in_=f_d[0:128, r0:r0 + TR, :])
                ft_b = fpool.tile([128, S], BF16, tag="fb", name="fb")
                nc.gpsimd.dma_start(out=ft_b[0:64, :], in_=f_d[128:192, r0:r0 + TR, :])
                nc.gpsimd.dma_start(out=ft_b[64:128, :], in_=f_d[128:192, r0:r0 + TR, :])

                # ---- qkv matmul (5 blocks, 10 halo rows) + psum->sbuf ----
                sb = []
                sb2 = []
                ms = 0
                for bi, psz in enumerate(BLK):
                    q_sb = qkvpool.tile([psz, PR * PW], BF16, tag=f"sb{bi}",
                                        name=f"sb{bi}")
                    q3 = q_sb.rearrange("p (r w) -> p r w", w=PW)
                    sb.append(q3)
                    if bi in QK_BLOCKS:
                        q_sb2 = qkvpool.tile([psz, PR * PW], BF16, tag=f"sc{bi}",
                                             name=f"sc{bi}")
                        q32 = q_sb2.rearrange("p (r w) -> p r w", w=PW)
                        sb2.append(q32)
                        nc.vector.memset(q32[:, :, 0:1], 0.0)
                        nc.vector.memset(q32[:, :, 129:130], 0.0)
                    else:
                        sb2.append(None)
                        nc.vector.memset(q3[:, :, 1:2], 0.0)
                        nc.vector.memset(q3[:, :, 130:131], 0.0)
                    for c0, csz in ((0, 4), (4, 4), (8, 2)):
                        ps = mmpsum.tile([psz, 4 * W], F32, tag="mmps",
                                         name="mmps", bufs=3)
                        nc.tensor.matmul(
                            ps[:, :csz * W],
                            wq_sb[0][:, ms:ms + psz],
                            xt[0][:, c0 * W:(c0 + csz) * W],
                            start=True, stop=False)
                        nc.tensor.matmul(
                            ps[:, :csz * W],
                            wq_sb[1][:, ms:ms + psz],
                            xt[1][:, c0 * W:(c0 + csz) * W],
                            start=False, stop=True)
                        ps3 = ps[:, :csz * W].rearrange("p (r w) -> p r w", w=W)
                        nc.scalar.activation(
                            q3[:, c0:c0 + csz, 2:2 + W], ps3, AF.Copy)
                        if bi in QK_BLOCKS:
                            nc.scalar.activation(
                                q32[:, c0:c0 + csz, 1:1 + W], ps3, AF.Copy)
                    ms += psz

                # previous tile's tail goes after this tile's qkv matmuls
                if prev is not None:
                    emit_tail(*prev)

                # ---- q,k stencil on DVE ----
                st = []
                for bi in QK_BLOCKS:
                    psz = BLK[bi]
                    acc = stpool.tile([psz, S], BF16, tag=f"st{bi}", name=f"st{bi}")
                    acc_ap = acc.rearrange("p (r w) -> p r w", w=W)
                    for idx, (ti, dy, dx) in enumerate(TAP_OFF):
                        if dx == 0:
                            src = sb[bi][:, 1 + dy:1 + dy + TR, 2:2 + W]
                        else:
                            src = sb2[bi][:, 1 + dy:1 + dy + TR,
                                          1 + dx:1 + dx + W]
                        w_ap = taps_sb[bi][:, ti:ti + 1]
                        if idx == 0:
                            nc.vector.tensor_scalar_mul(acc_ap, src, w_ap)
                        else:
                            nc.vector.scalar_tensor_tensor(
                                acc_ap, src, w_ap, acc_ap,
                                op0=AL.mult, op1=AL.add)
                    st.append(acc)

                # ---- qf/kf multiply (in place) + squares ----
                fts = [ft_a, ft_b, ft_a]
                for i, bi in enumerate(QK_BLOCKS):
                    nc.vector.tensor_mul(st[i][:], st[i][:], fts[i][:])
                    scr = scrpool.tile([128, S], BF16, tag=f"scr{i}",
                                       name=f"scr{i}")
                    nc.scalar.activation(
                        scr[:], st[i][:], AF.Square,
                        accum_out=sq_sb[i][:, t:t + 1])

                prev = (t, st, sb)

            emit_tail(*prev)

            # ================= final small-tensor phase =================
            rb = []
            for i in range(3):
                sq1 = finpool.tile([128, 1], F32, tag=f"sq1_{i}", name=f"sq1_{i}")
                nc.vector.tensor_reduce(
                    sq1[:], sq_sb[i][:], axis=mybir.AxisListType.X, op=AL.add)
                nc.vector.tensor_scalar_max(sq1[:], sq1[:], 1e-24)
                nq = finpool.tile([128, 1], F32, tag=f"nq_{i}", name=f"nq_{i}")
                nc.scalar.activation(nq[:], sq1[:], AF.Sqrt)
                r = finpool.tile([128, 1], F32, tag=f"rq_{i}", name=f"rq_{i}")
                nc.vector.reciprocal(r[:], nq[:])
                rb.append(r)

            G_sb = [finpool.tile([128, C], F32, tag="G0", name="G0"),
                    finpool.tile([64, C], F32, tag="G1", name="G1")]
            nc.vector.tensor_scalar_mul(G_sb[0][:], g_ps[0], rb[0][:])
            nc.vector.tensor_scalar_mul(G_sb[1][:], g_ps[1], rb[1][0:64, :])

            gt0_t = vpsum.tile([128, 512], F32, tag="vp3", name="gt0")
            gt1_t = vpsum.tile([64, 512], F32, tag="vp4", name="gt1")
            gt_ps = [gt0_t[:, 0:C], gt1_t[:, 0:C]]
            nc.tensor.matmul(gt_ps[0][:, 0:128], G_sb[0][:, 0:128], identf[:],
                             is_transpose=True, start=True, stop=True)
            nc.tensor.matmul(gt_ps[0][:, 128:192], G_sb[1][:, 0:128],
                             identf[0:64, 0:64], is_transpose=True,
                             start=True, stop=True)
            nc.tensor.matmul(gt_ps[1][:, 0:128], G_sb[0][:, 128:192], identf[:],
                             is_transpose=True, start=True, stop=True)
            nc.tensor.matmul(gt_ps[1][:, 128:192], G_sb[1][:, 128:192],
                             identf[0:64, 0:64], is_transpose=True,
                             start=True, stop=True)

            rkp = finpool.tile([32, HEADS], F32, tag="rkp", name="rkp")
            nc.vector.memset(rkp[:], 0.0)
            for h in range(HEADS):
                a0 = h * CH
                a1 = a0 + CH
                if a1 <= 128:
                    nc.sync.dma_start(out=rkp[0:CH, h:h + 1],
                                      in_=rb[2][a0:a1, :])
                elif a0 >= 128:
                    nc.sync.dma_start(out=rkp[0:CH, h:h + 1],
                                      in_=rb[1][64 + a0 - 128:64 + a1 - 128, :])
                else:
                    m = 128 - a0
                    nc.sync.dma_start(out=rkp[0:m, h:h + 1],
                                      in_=rb[2][a0:128, :])
                    nc.sync.dma_start(out=rkp[m:CH, h:h + 1],
                                      in_=rb[1][64:64 + a1 - 128, :])
            nc.vector.tensor_mul(rkp[0:CH, :], rkp[0:CH, :], temp_sb[:])

            gt_sb = [finpool.tile([128, C], F32, tag="gts0", name="gts0"),
                     finpool.tile([64, C], F32, tag="gts1", name="gts1")]
            nc.vector.tensor_copy(gt_sb[0][:], gt_ps[0])
            nc.vector.tensor_copy(gt_sb[1][:], gt_ps[1])
            at = finpool.tile([32, HEADS * 32], F32, tag="at", name="at")
            nc.vector.memset(at[:], 0.0)
            for h in range(HEADS):
                a0 = h * CH
                a1 = a0 + CH
                col = slice(a0, a1)
                if a1 <= 128:
                    nc.sync.dma_start(out=at[0:CH, h * 32:h * 32 + CH],
                                      in_=gt_sb[0][a0:a1, col])
                elif a0 >= 128:
                    nc.sync.dma_start(out=at[0:CH, h * 32:h * 32 + CH],
                                      in_=gt_sb[1][a0 - 128:a1 - 128, col])
                else:
                    m = 128 - a0
                    nc.sync.dma_start(out=at[0:m, h * 32:h * 32 + CH],
                                      in_=gt_sb[0][a0:128, col])
                    nc.sync.dma_start(out=at[m:CH, h * 32:h * 32 + CH],
                                      in_=gt_sb[1][0:a1 - 128, col])
                nc.vector.tensor_scalar_mul(
                    at[0:CH, h * 32:h * 32 + CH],
                    at[0:CH, h * 32:h * 32 + CH],
                    rkp[0:CH, h:h + 1])

            a_sb = finpool.tile([32, HEADS * 32], F32, tag="a", name="a")
            nc.vector.transpose(a_sb[:], at[:])
            e_sb = finpool.tile([32, HEADS * 32], F32, tag="e", name="e")
            nc.scalar.activation(e_sb[:], a_sb[:], AF.Exp)
            e3 = e_sb.rearrange("p (h d) -> p h d", d=32)
            sums = finpool.tile([CH, HEADS], F32, tag="sums", name="sums")
            nc.vector.tensor_reduce(
                sums[:], e3[0:CH, :, 0:CH], axis=mybir.AxisListType.X, op=AL.add)
            rs = finpool.tile([CH, HEADS], F32, tag="rs", name="rs")
            nc.vector.reciprocal(rs[:], sums[:])
            attn = finpool.tile([CH, HEADS * CH], BF16, tag="attn", name="attn")
            for h in range(HEADS):
                nc.vector.tensor_scalar_mul(
                    attn[:, h * CH:(h + 1) * CH],
                    e_sb[0:CH, h * 32:h * 32 + CH],
                    rs[:, h:h + 1])

            bd = [finpool.tile([96, C], BF16, tag="bd0", name="bd0"),
                  finpool.tile([96, C], BF16, tag="bd1", name="bd1")]
            nc.vector.memset(bd[0][:], 0.0)
            nc.vector.memset(bd[1][:], 0.0)
            for h in range(HEADS):
                nc.sync.dma_start(
                    out=bd[h // 4][(h % 4) * CH:(h % 4) * CH + CH,
                                   h * CH:(h + 1) * CH],
                    in_=attn[:, h * CH:(h + 1) * CH])
            mt0_t = tpsum.tile([128, C], F32, tag="qt", name="mt0")
            mt1_t = tpsum.tile([128, C], F32, tag="kt", name="mt1")
            mt_ps = [mt0_t[:, :], mt1_t[0:64, :]]
            for mi, msl in enumerate((slice(0, 128), slice(128, 192))):
                for k in range(2):
                    nc.tensor.matmul(mt_ps[mi], bd[k][:, msl], wp_sb[k][:],
                                     start=(k == 0), stop=(k == 1))
            mt_sb = [finpool.tile([128, C], BF16, tag="mt_sb0", name="mt_sb0"),
                     finpool.tile([64, C], BF16, tag="mt_sb1", name="mt_sb1")]
            nc.vector.tensor_copy(mt_sb[0][:], mt_ps[0])
            nc.vector.tensor_copy(mt_sb[1][:], mt_ps[1])

            # group chunks 3-at-a-time per stationary so the PE streams
            # same-weight matmul runs back-to-back instead of reloading
            # weights (and going cold) on every single MM
            for mi, (msz, msl) in enumerate(((128, slice(0, 128)),
                                             (64, slice(128, 192)))):
                for jg in range(0, N // 512, 3):
                    js = list(range(jg, min(jg + 3, N // 512)))
                    pss = [mmpsum.tile([msz, 512], F32, tag="mmps",
                                       name="mmps", bufs=3) for _ in js]
                    for k in range(2):
                        for ji, j in enumerate(js):
                            col = slice(j * 512, (j + 1) * 512)
                            nc.tensor.matmul(
                                pss[ji][:], mt_sb[k][:, msl], v_sb[k][:, col],
                                start=(k == 0), stop=(k == 1))
                    for ji, j in enumerate(js):
                        col = slice(j * 512, (j + 1) * 512)
                        osb = outpool.tile([msz, 512], BF16, tag=f"osb{mi}",
                                           name=f"osb{mi}")
                        # split drain copies between ScalarE and idle DVE
                        if j % 2 == 0:
                            nc.scalar.activation(osb[:], pss[ji][:], AF.Copy)
                        else:
                            nc.vector.tensor_copy(osb[:], pss[ji][:])
                        cs = 0 if mi == 0 else 128
                        nc.sync.dma_start(out=out_d[cs:cs + msz, col],
                                          in_=osb[:])
    nc.finalize()
    return nc


_NC_CACHE = {}


def _perm():
    return (list(range(0, 128)) + list(range(128, 192))
            + list(range(320, 384)) + list(range(192, 320))
            + list(range(384, 576)))


def kernel(x, feature, W_qkv, W_dw, W_proj, temperature):
    import ml_dtypes
    b = x.shape[0]
    perm = _perm()
    wq_p = np.asarray(W_qkv, np.float32)[perm, :]
    wq = np.ascontiguousarray(wq_p.T).astype(ml_dtypes.bfloat16)
    taps = np.ascontiguousarray(
        np.asarray(W_dw, np.float32).reshape(O, 9)[perm, :])
    vtaps = taps[384:576, :]
    vd3 = np.zeros((128, 9 * 128), np.float32)
    for ti in range(9):
        vd3[:, ti * 128:(ti + 1) * 128][np.arange(128), np.arange(128)] = \
            vtaps[0:128, ti]
    vd4 = np.zeros((64, 9 * 64), np.float32)
    for ti in range(9):
        vd4[:, ti * 64:(ti + 1) * 64][np.arange(64), np.arange(64)] = \
            vtaps[128:192, ti]
    wp = np.ascontiguousarray(np.asarray(W_proj, np.float32).T).astype(
        ml_dtypes.bfloat16)
    temp = np.broadcast_to(
        np.asarray(temperature, np.float32).reshape(1, HEADS), (CH, HEADS))
    temp = np.ascontiguousarray(temp)

    if "nc" not in _NC_CACHE:
        _NC_CACHE["nc"] = build_nc()
    nc = _NC_CACHE["nc"]

    in_maps = []
    for i in range(b):
        in_maps.append({
            "x": np.ascontiguousarray(np.asarray(x[i], np.float32)),
            "f": np.ascontiguousarray(np.asarray(feature[i], np.float32)),
            "wq": wq, "taps": taps,
            "vdiag3": vd3.astype(ml_dtypes.bfloat16),
            "vdiag4": vd4.astype(ml_dtypes.bfloat16),
            "wp": wp, "temp": temp,
            "identb": np.eye(128, dtype=np.float32).astype(ml_dtypes.bfloat16),
            "identf": np.eye(128, dtype=np.float32),
        })
    res = run_bass_kernel_spmd(nc, in_maps, list(range(b)))
    outs = [np.asarray(r["out"], np.float32).reshape(C, H, W)
            for r in res.results]
    return np.stack(outs, axis=0)


# revision 34
# speedup vs baseline: 1.0282x; 1.0282x over previous
"""AttentionV1 Trainium2 Bass kernel (v5 = v2 + TS/TT stencil + fused casts).

Data-parallel over batch: 8 images -> 8 NeuronCores. Per core:
  qkv = W_qkv @ x            (1x1 conv, PE, bf16, permuted 5-block layout)
  qkv = dwconv3x3(qkv)       (q,k: tensor_scalar products + tensor_tensor
                              adds on DVE; v: diag-matmul on PE)
  qf = q*f, kf = k*f         (DVE)
  G  = qf @ kf^T             (PE transpose-via-identity + PE gram)
  attn = softmax(G / (nq nk^T))  (small-tensor phase)
  out = (blockdiag(attn)^T @ W_proj^T)^T @ v   (PE)

Channel blocks (output-channel permutation of W_qkv/taps):
  B0 = q[0:128], B1 = q[128:192] || k[128:192], B2 = k[0:128],
  B3 = v[0:128], B4 = v[128:192]
"""
import sys

for _p in ("/opt/trn_rl_repo",):
    if _p not in sys.path:
        sys.path.insert(0, _p)

import numpy as np

import concourse.bass as bass
import concourse.bacc as bacc
import concourse.mybir as mybir
from concourse.tile import TileContext
from concourse.bass_utils import run_bass_kernel_spmd

F32 = mybir.dt.float32
BF16 = mybir.dt.bfloat16
AL = mybir.AluOpType
AF = mybir.ActivationFunctionType

C = 192          # channels
O = 576          # 3*C
H = 128
W = 128
N = H * W        # 16384
HEADS = 8
CH = 24          # channels per head
TR = 8           # rows per spatial tile
NT = H // TR     # 16 tiles
S = TR * W       # 1024 spatial elems per tile
PR = TR + 2      # padded rows (halo)
PW = W + 4       # padded width: cols [2,130) hold x in [0,128)
NCHUNK = S // 128  # 8 transpose chunks per tile

BLK = [128, 128, 128, 128, 64]
QK_BLOCKS = (0, 1, 2)
TAP_OFF = [(3 * (dy + 1) + (dx + 1), dy, dx)
           for dy in (-1, 0, 1) for dx in (-1, 0, 1)]
DVE_TAPS = [(1, -1, 0)] + [t for t in TAP_OFF if t[0] != 1]


def build_nc():
    nc = bacc.Bacc()
    x_d = nc.declare_dram_parameter("x", [C, H, W], F32, isOutput=False)
    f_d = nc.declare_dram_parameter("f", [C, H, W], F32, isOutput=False)
    wq_d = nc.declare_dram_parameter("wq", [C, O], BF16, isOutput=False)
    taps_d = nc.declare_dram_parameter("taps", [O, 9], F32, isOutput=False)
    vd3_d = nc.declare_dram_parameter("vdiag3", [128, 9 * 128], BF16, isOutput=False)
    vd4_d = nc.declare_dram_parameter("vdiag4", [64, 9 * 64], BF16, isOutput=False)
    wp_d = nc.declare_dram_parameter("wp", [C, C], BF16, isOutput=False)
    temp_d = nc.declare_dram_parameter("temp", [CH, HEADS], F32, isOutput=False)
    idb_d = nc.declare_dram_parameter("identb", [128, 128], BF16, isOutput=False)
    idf_d = nc.declare_dram_parameter("identf", [128, 128], F32, isOutput=False)
    out_d = nc.declare_dram_parameter("out", [C, N], BF16, isOutput=True)

    with TileContext(nc) as tc:
        with (
            tc.tile_pool(name="const", bufs=1) as cpool,
            tc.tile_pool(name="vstore", bufs=1) as vpool,
            tc.tile_pool(name="xin", bufs=3) as xpool,
            tc.tile_pool(name="fin", bufs=2) as fpool,
            tc.tile_pool(name="qkv", bufs=3) as qkvpool,
            tc.tile_pool(name="st", bufs=3) as stpool,
            tc.tile_pool(name="scr", bufs=2) as scrpool,
            tc.tile_pool(name="tsb", bufs=3) as tsbpool,
            tc.tile_pool(name="fin2", bufs=1) as finpool,
            tc.tile_pool(name="outsb", bufs=3) as outpool,
            tc.tile_pool(name="mm", bufs=2, space="PSUM") as mmpsum,
            tc.tile_pool(name="vps", bufs=1, space="PSUM") as vpsum,
            tc.tile_pool(name="tps", bufs=1, space="PSUM") as tpsum,
            tc.tile_pool(name="gram", bufs=1, space="PSUM") as gpsum,
        ):
            # ---- constants ----
            wq_sb = [cpool.tile([128, O], BF16, tag="wq0", name="wq0"),
                     cpool.tile([64, O], BF16, tag="wq1", name="wq1")]
            nc.sync.dma_start(out=wq_sb[0][:], in_=wq_d[0:128, :])
            nc.sync.dma_start(out=wq_sb[1][:], in_=wq_d[128:192, :])
            taps_sb = []
            ms = 0
            for bi, psz in enumerate(BLK):
                tt = cpool.tile([psz, 9], F32, tag=f"taps{bi}", name=f"taps{bi}")
                nc.sync.dma_start(out=tt[:], in_=taps_d[ms:ms + psz, :])
                taps_sb.append(tt)
                ms += psz
            vd3 = cpool.tile([128, 9 * 128], BF16, tag="vd3", name="vd3")
            nc.sync.dma_start(out=vd3[:], in_=vd3_d[:])
            vd4 = cpool.tile([64, 9 * 64], BF16, tag="vd4", name="vd4")
            nc.sync.dma_start(out=vd4[:], in_=vd4_d[:])
            wp_sb = [cpool.tile([96, C], BF16, tag="wp0", name="wp0"),
                     cpool.tile([96, C], BF16, tag="wp1", name="wp1")]
            nc.sync.dma_start(out=wp_sb[0][:], in_=wp_d[0:96, :])
            nc.sync.dma_start(out=wp_sb[1][:], in_=wp_d[96:192, :])
            temp_sb = cpool.tile([CH, HEADS], F32, tag="temp", name="temp")
            nc.sync.dma_start(out=temp_sb[:], in_=temp_d[:])
            identb = cpool.tile([128, 128], BF16, tag="identb", name="identb")
            nc.sync.dma_start(out=identb[:], in_=idb_d[:])
            identf = cpool.tile([128, 128], F32, tag="identf", name="identf")
            nc.sync.dma_start(out=identf[:], in_=idf_d[:])

            v_sb = [vpool.tile([128, N], BF16, tag="v0", name="v0"),
                    vpool.tile([64, N], BF16, tag="v1", name="v1")]
            sq_sb = [cpool.tile([128, NT], F32, tag=f"sq{i}", name=f"sq{i}")
                     for i in range(3)]
            # gram accumulators packed into one PSUM bank
            g_all = gpsum.tile([128, 512], F32, tag="g", name="g")
            g_ps = [g_all[:, 0:C], g_all[0:64, 256:256 + C]]

            def emit_tail(t, st, sb):
                """Transposes + gram + v-stencil for tile t. Emitted AFTER
                tile t+1's qkv matmuls so the PE's in-order queue overlaps
                this (DVE/ACT-gated) work with the next tile's independent
                qkv streams."""
                vps = {}
                for j in range(NCHUNK):
                    g = t * NCHUNK + j
                    col = slice(j * 128, (j + 1) * 128)
                    qt_ps = tpsum.tile([128, C], F32, tag="qt", name="qt")
                    kt_ps = tpsum.tile([128, C], F32, tag="kt", name="kt")
                    nc.tensor.matmul(qt_ps[:, 0:128], st[0][:, col],
                                     identb[:], start=True, stop=True)
                    nc.tensor.matmul(qt_ps[:, 128:192], st[1][0:64, col],
                                     identb[0:64, 0:64], start=True, stop=True)
                    nc.tensor.matmul(kt_ps[:, 0:128], st[2][:, col],
                                     identb[:], start=True, stop=True)
                    nc.tensor.matmul(kt_ps[:, 128:192], st[1][64:128, col],
                                     identb[64:128, 64:128], start=True, stop=True)
                    qt_sb = tsbpool.tile([128, C], BF16, tag="qts", name="qts")
                    kt_sb = tsbpool.tile([128, C], BF16, tag="kts", name="kts")
                    nc.scalar.activation(qt_sb[:], qt_ps[:], AF.Copy)
                    nc.scalar.activation(kt_sb[:], kt_ps[:], AF.Copy)
                    half = j // 4
                    jj = j % 4
                    if jj == 0:
                        vps["vp3"] = vpsum.tile([128, 512], F32, tag="vp3",
                                                name="vp3")
                        vps["vp4"] = vpsum.tile([64, 512], F32, tag="vp4",
                                                name="vp4")
                    tap_sl = ((0, 2), (2, 4), (4, 6), (6, 9))[jj]
                    for ti in range(tap_sl[0], tap_sl[1]):
                        _, dy, dx = TAP_OFF[ti]
                        r_lo = 1 + dy + 4 * half
                        rhs3 = sb[3][:, r_lo:r_lo + 4, 2 + dx:2 + dx + W]
                        nc.tensor.matmul(
                            vps["vp3"][:], vd3[:, ti * 128:(ti + 1) * 128],
                            rhs3, start=(ti == 0), stop=(ti == 8))
                        rhs4 = sb[4][:, r_lo:r_lo + 4, 2 + dx:2 + dx + W]
                        nc.tensor.matmul(
                            vps["vp4"][:], vd4[:, ti * 64:(ti + 1) * 64],
                            rhs4, start=(ti == 0), stop=(ti == 8))
                    if jj == 3:
                        cdst = slice(t * S + half * 512,
                                     t * S + (half + 1) * 512)
                        nc.scalar.activation(v_sb[0][:, cdst], vps["vp3"][:],
                                             AF.Copy)
                        nc.scalar.activation(v_sb[1][:, cdst], vps["vp4"][:],
                                             AF.Copy)
                    nc.tensor.matmul(
                        g_ps[0], qt_sb[:, 0:128], kt_sb[:],
                        start=(g == 0), stop=(g == NT * NCHUNK - 1))
                    nc.tensor.matmul(
                        g_ps[1], qt_sb[:, 128:192], kt_sb[:],
                        start=(g == 0), stop=(g == NT * NCHUNK - 1))

            prev = None
            for t in range(NT):
                r0 = t * TR
                xt = [xpool.tile([128, PR * W], BF16, tag="x0", name="x0"),
                      xpool.tile([64, PR * W], BF16, tag="x1", name="x1")]
                lo = r0 - 1
                hi = r0 + TR + 1
                dlo = max(lo, 0)
                dhi = min(hi, H)
                off = dlo - lo
                for ci, (cs, cp) in enumerate(((0, 128), (128, 64))):
                    if lo < 0:
                        nc.vector.memset(xt[ci][:, 0:W], 0.0)
                    if hi > H:
                        nc.vector.memset(xt[ci][:, (PR - 1) * W:PR * W], 0.0)
                    nc.gpsimd.dma_start(
                        out=xt[ci][:, off * W:(off + dhi - dlo) * W],
                        in_=x_d[cs:cs + cp, dlo:dhi, :],
                    )
                ft_a = fpool.tile([128, S], BF16, tag="fa", name="fa")
                nc.gpsimd.dma_start(out=ft_a[:], in_=f_d[0:128, r0:r0 + TR, :])
                ft_b = fpool.tile([128, S], BF16, tag="fb", name="fb")
                nc.gpsimd.dma_start(out=ft_b[0:64, :], in_=f_d[128:192, r0:r0 + TR, :])
                nc.gpsimd.dma_start(out=ft_b[64:128, :], in_=f_d[128:192, r0:r0 + TR, :])

                # ---- qkv matmul (5 blocks, 10 halo rows) + psum->sbuf ----
                sb = []
                sb2 = []
                ms = 0
                for bi, psz in enumerate(BLK):
                    q_sb = qkvpool.tile([psz, PR * PW], BF16, tag=f"sb{bi}",
                                        name=f"sb{bi}")
                    q3 = q_sb.rearrange("p (r w) -> p r w", w=PW)
                    sb.append(q3)
                    if bi in QK_BLOCKS:
                        q_sb2 = qkvpool.tile([psz, PR * PW], BF16, tag=f"sc{bi}",
                                             name=f"sc{bi}")
                        q32 = q_sb2.rearrange("p (r w) -> p r w", w=PW)
                        sb2.append(q32)
                        nc.vector.memset(q32[:, :, 0:1], 0.0)
                        nc.vector.memset(q32[:, :, 129:130], 0.0)
                    else:
                        sb2.append(None)
                        nc.vector.memset(q3[:, :, 1:2], 0.0)
                        nc.vector.memset(q3[:, :, 130:131], 0.0)
                    for c0, csz in ((0, 4), (4, 4), (8, 2)):
                        ps = mmpsum.tile([psz, 4 * W], F32, tag="mmps",
                                         name="mmps", bufs=3)
                        nc.tensor.matmul(
                            ps[:, :csz * W],
                            wq_sb[0][:, ms:ms + psz],
                            xt[0][:, c0 * W:(c0 + csz) * W],
                            start=True, stop=False)
                        nc.tensor.matmul(
                            ps[:, :csz * W],
                            wq_sb[1][:, ms:ms + psz],
                            xt[1][:, c0 * W:(c0 + csz) * W],
                            start=False, stop=True)
                        ps3 = ps[:, :csz * W].rearrange("p (r w) -> p r w", w=W)
                        nc.scalar.activation(
                            q3[:, c0:c0 + csz, 2:2 + W], ps3, AF.Copy)
                        if bi in QK_BLOCKS:
                            nc.scalar.activation(
                                q32[:, c0:c0 + csz, 1:1 + W], ps3, AF.Copy)
                    ms += psz

                # previous tile's tail goes after this tile's qkv matmuls
                if prev is not None:
                    emit_tail(*prev)

                # ---- q,k stencil on DVE ----
                st = []
                for bi in QK_BLOCKS:
                    psz = BLK[bi]
                    acc = stpool.tile([psz, S], BF16, tag=f"st{bi}", name=f"st{bi}")
                    acc_ap = acc.rearrange("p (r w) -> p r w", w=W)
                    for idx, (ti, dy, dx) in enumerate(TAP_OFF):
                        if dx == 0:
                            src = sb[bi][:, 1 + dy:1 + dy + TR, 2:2 + W]
                        else:
                            src = sb2[bi][:, 1 + dy:1 + dy + TR,
                                          1 + dx:1 + dx + W]
                        w_ap = taps_sb[bi][:, ti:ti + 1]
                        if idx == 0:
                            nc.vector.tensor_scalar_mul(acc_ap, src, w_ap)
                        else:
                            nc.vector.scalar_tensor_tensor(
                                acc_ap, src, w_ap, acc_ap,
                                op0=AL.mult, op1=AL.add)
                    st.append(acc)

                # ---- qf/kf multiply (in place) + squares ----
                fts = [ft_a, ft_b, ft_a]
                for i, bi in enumerate(QK_BLOCKS):
                    nc.vector.tensor_mul(st[i][:], st[i][:], fts[i][:])
                    scr = scrpool.tile([128, S], BF16, tag=f"scr{i}",
                                       name=f"scr{i}")
                    nc.scalar.activation(
                        scr[:], st[i][:], AF.Square,
                        accum_out=sq_sb[i][:, t:t + 1])

                prev = (t, st, sb)

            emit_tail(*prev)

            # ================= final small-tensor phase =================
            rb = []
            for i in range(3):
                sq1 = finpool.tile([128, 1], F32, tag=f"sq1_{i}", name=f"sq1_{i}")
                nc.vector.tensor_reduce(
                    sq1[:], sq_sb[i][:], axis=mybir.AxisListType.X, op=AL.add)
                nc.vector.tensor_scalar_max(sq1[:], sq1[:], 1e-24)
                nq = finpool.tile([128, 1], F32, tag=f"nq_{i}", name=f"nq_{i}")
                nc.scalar.activation(nq[:], sq1[:], AF.Sqrt)
                r = finpool.tile([128, 1], F32, tag=f"rq_{i}", name=f"rq_{i}")
                nc.vector.reciprocal(r[:], nq[:])
                rb.append(r)

            G_sb = [finpool.tile([128, C], F32, tag="G0", name="G0"),
                    finpool.tile([64, C], F32, tag="G1", name="G1")]
            nc.vector.tensor_scalar_mul(G_sb[0][:], g_ps[0], rb[0][:])
            nc.vector.tensor_scalar_mul(G_sb[1][:], g_ps[1], rb[1][0:64, :])

            gt0_t = vpsum.tile([128, 512], F32, tag="vp3", name="gt0")
            gt1_t = vpsum.tile([64, 512], F32, tag="vp4", name="gt1")
            gt_ps = [gt0_t[:, 0:C], gt1_t[:, 0:C]]
            nc.tensor.matmul(gt_ps[0][:, 0:128], G_sb[0][:, 0:128], identf[:],
                             is_transpose=True, start=True, stop=True)
            nc.tensor.matmul(gt_ps[0][:, 128:192], G_sb[1][:, 0:128],
                             identf[0:64, 0:64], is_transpose=True,
                             start=True, stop=True)
            nc.tensor.matmul(gt_ps[1][:, 0:128], G_sb[0][:, 128:192], identf[:],
                             is_transpose=True, start=True, stop=True)
            nc.tensor.matmul(gt_ps[1][:, 128:192], G_sb[1][:, 128:192],
                             identf[0:64, 0:64], is_transpose=True,
                             start=True, stop=True)

            rkp = finpool.tile([32, HEADS], F32, tag="rkp", name="rkp")
            nc.vector.memset(rkp[:], 0.0)
            for h in range(HEADS):
                a0 = h * CH
                a1 = a0 + CH
                if a1 <= 128:
                    nc.sync.dma_start(out=rkp[0:CH, h:h + 1],
                                      in_=rb[2][a0:a1, :])
                elif a0 >= 128:
                    nc.sync.dma_start(out=rkp[0:CH, h:h + 1],
                                      in_=rb[1][64 + a0 - 128:64 + a1 - 128, :])
                else:
                    m = 128 - a0
                    nc.sync.dma_start(out=rkp[0:m, h:h + 1],
                                      in_=rb[2][a0:128, :])
                    nc.sync.dma_start(out=rkp[m:CH, h:h + 1],
                                      in_=rb[1][64:64 + a1 - 128, :])
            nc.vector.tensor_mul(rkp[0:CH, :], rkp[0:CH, :], temp_sb[:])

            gt_sb = [finpool.tile([128, C], F32, tag="gts0", name="gts0"),
                     finpool.tile([64, C], F32, tag="gts1", name="gts1")]
            nc.vector.tensor_copy(gt_sb[0][:], gt_ps[0])
            nc.vector.tensor_copy(gt_sb[1][:], gt_ps[1])
            at = finpool.tile([32, HEADS * 32], F32, tag="at", name="at")
            nc.vector.memset(at[:], 0.0)
            for h in range(HEADS):
                a0 = h * CH
                a1 = a0 + CH
                col = slice(a0, a1)
                if a1 <= 128:
                    nc.sync.dma_start(out=at[0:CH, h * 32:h * 32 + CH],
                                      in_=gt_sb[0][a0:a1, col])
                elif a0 >= 128:
                    nc.sync.dma_start(out=at[0:CH, h * 32:h * 32 + CH],
                                      in_=gt_sb[1][a0 - 128:a1 - 128, col])
                else:
                    m = 128 - a0
                    nc.sync.dma_start(out=at[0:m, h * 32:h * 32 + CH],
                                      in_=gt_sb[0][a0:128, col])
                    nc.sync.dma_start(out=at[m:CH, h * 32:h * 32 + CH],
                                      in_=gt_sb[1][0:a1 - 128, col])
                nc.vector.tensor_scalar_mul(
                    at[0:CH, h * 32:h * 32 + CH],
                    at[0:CH, h * 32:h * 32 + CH],
                    rkp[0:CH, h:h + 1])

            a_sb = finpool.tile([32, HEADS * 32], F32, tag="a", name="a")
            nc.vector.transpose(a_sb[:], at[:])
            e_sb = finpool.tile([32, HEADS * 32], F32, tag="e", name="e")
            nc.scalar.activation(e_sb[:], a_sb[:], AF.Exp)
            e3 = e_sb.rearrange("p (h d) -> p h d", d=32)
            sums = finpool.tile([CH, HEADS], F32, tag="sums", name="sums")
            nc.vector.tensor_reduce(
                sums[:], e3[0:CH, :, 0:CH], axis=mybir.AxisListType.X, op=AL.add)
            rs = finpool.tile([CH, HEADS], F32, tag="rs", name="rs")
            nc.vector.reciprocal(rs[:], sums[:])
            attn = finpool.tile([CH, HEADS * CH], BF16, tag="attn", name="attn")
            for h in range(HEADS):
                nc.vector.tensor_scalar_mul(
                    attn[:, h * CH:(h + 1) * CH],
                    e_sb[0:CH, h * 32:h * 32 + CH],
                    rs[:, h:h + 1])

            bd = [finpool.tile([96, C], BF16, tag="bd0", name="bd0"),
                  finpool.tile([96, C], BF16, tag="bd1", name="bd1")]
            nc.vector.memset(bd[0][:], 0.0)
            nc.vector.memset(bd[1][:], 0.0)
            for h in range(HEADS):
                nc.sync.dma_start(
                    out=bd[h // 4][(h % 4) * CH:(h % 4) * CH + CH,
                                   h * CH:(h + 1) * CH],
                    in_=attn[:, h * CH:(h + 1) * CH])
            mt0_t = tpsum.tile([128, C], F32, tag="qt", name="mt0")
            mt1_t = tpsum.tile([128, C], F32, tag="kt", name="mt1")
            mt_ps = [mt0_t[:, :], mt1_t[0:64, :]]
            for mi, msl in enumerate((slice(0, 128), slice(128, 192))):
                for k in range(2):
                    nc.tensor.matmul(mt_ps[mi], bd[k][:, msl], wp_sb[k][:],
                                     start=(k == 0), stop=(k == 1))
            mt_sb = [finpool.tile([128, C], BF16, tag="mt_sb0", name="mt_sb0"),
                     finpool.tile([64, C], BF16, tag="mt_sb1", name="mt_sb1")]
            nc.vector.tensor_copy(mt_sb[0][:], mt_ps[0])
            nc.vector.tensor_copy(mt_sb[1][:], mt_ps[1])

            for j in range(N // 512):
                col = slice(j * 512, (j + 1) * 512)
                for mi, (msz, msl) in enumerate(((128, slice(0, 128)),
                                                 (64, slice(128, 192)))):
                    ps = mmpsum.tile([msz, 512], F32, tag="mmps", name="mmps",
                                     bufs=3)
                    nc.tensor.matmul(ps[:], mt_sb[0][:, msl], v_sb[0][:, col],
                                     start=True, stop=False)
                    nc.tensor.matmul(ps[:], mt_sb[1][:, msl], v_sb[1][:, col],
                                     start=False, stop=True)
                    osb = outpool.tile([msz, 512], BF16, tag=f"osb{mi}",
                                       name=f"osb{mi}")
                    # alternate copies between ScalarE and the otherwise-idle
                    # DVE so the output phase drains twice as fast
                    if j % 2 == 0:
                        nc.scalar.activation(osb[:], ps[:], AF.Copy)
                    else:
                        nc.vector.tensor_copy(osb[:], ps[:])
                    cs = 0 if mi == 0 else 128
                    nc.sync.dma_start(out=out_d[cs:cs + msz, col], in_=osb[:])
    nc.finalize()
    return nc


_NC_CACHE = {}


def _perm():
    return (list(range(0, 128)) + list(range(128, 192))
            + list(range(320, 384)) + list(range(192, 320))
            + list(range(384, 576)))


def kernel(x, feature, W_qkv, W_dw, W_proj, temperature):
    import ml_dtypes
    b = x.shape[0]
    perm = _perm()
    wq_p = np.asarray(W_qkv, np.float32)[perm, :]
    wq = np.ascontiguousarray(wq_p.T).astype(ml_dtypes.bfloat16)
    taps = np.ascontiguousarray(
        np.asarray(W_dw, np.float32).reshape(O, 9)[perm, :])
    vtaps = taps[384:576, :]
    vd3 = np.zeros((128, 9 * 128), np.float32)
    for ti in range(9):
        vd3[:, ti * 128:(ti + 1) * 128][np.arange(128), np.arange(128)] = \
            vtaps[0:128, ti]
    vd4 = np.zeros((64, 9 * 64), np.float32)
    for ti in range(9):
        vd4[:, ti * 64:(ti + 1) * 64][np.arange(64), np.arange(64)] = \
            vtaps[128:192, ti]
    wp = np.ascontiguousarray(np.asarray(W_proj, np.float32).T).astype(
        ml_dtypes.bfloat16)
    temp = np.broadcast_to(
        np.asarray(temperature, np.float32).reshape(1, HEADS), (CH, HEADS))
    temp = np.ascontiguousarray(temp)

    if "nc" not in _NC_CACHE:
        _NC_CACHE["nc"] = build_nc()
    nc = _NC_CACHE["nc"]

    in_maps = []
    for i in range(b):
        in_maps.append({
            "x": np.ascontiguousarray(np.asarray(x[i], np.float32)),
            "f": np.ascontiguousarray(np.asarray(feature[i], np.float32)),
            "wq": wq, "taps": taps,
            "vdiag3": vd3.astype(ml_dtypes.bfloat16),
            "vdiag4": vd4.astype(ml_dtypes.bfloat16),
            "wp": wp, "temp": temp,
            "identb": np.eye(128, dtype=np.float32).astype(ml_dtypes.bfloat16),
            "identf": np.eye(128, dtype=np.float32),
        })
    res = run_bass_kernel_spmd(nc, in_maps, list(range(b)))
    outs = [np.asarray(r["out"], np.float32).reshape(C, H, W)
            for r in res.results]
    return np.stack(outs, axis=0)
